# revision 26
# baseline (speedup 1.0000x reference)
"""Trainium2 Bass kernel for nn_AttentionModule_50002009260608.

B=16, C=512, H=W=24 (HW=576), TF=512, NH=8, CPH=64.
Data-parallel over batch: 2 batch elements per core x 8 cores.

Host folds Wv@Wm1 into one conv weight (vl is never materialized) and the
V bias passes through the softmax as a per-channel constant (att columns
sum to 2); together with the cross-attention value column it rides in
column 576 of the attention-output tiles, through the final conv, and is
added during the final eviction.

All matmuls bf16. Scores/convs/finals accumulate into 2-bank PSUM tiles
(cols 0:288 in bank A, 512:800 in bank B) so one strided activation /
eviction covers both halves. A generator-based scheduler weaves conv /
final matmul chunks between score matmuls at ~0.5us granularity to keep
the PE busy continuously (TRN2 runs PE at 2.4GHz only after 3us of
uninterrupted work, else 1.2GHz).

HW constraints found the hard way: Pool (gpsimd) cannot access PSUM;
engine APs must start at a partition multiple of 32; partition_broadcast
only sources partition 0 of a tile; reciprocal_approx_* crashes the
runtime. Hence the baseline-style softmax-denominator parking at
partitions 0/32/64/96 with one reciprocal per head-pair.
"""

import ml_dtypes
import numpy as np
from collections import deque
from contextlib import ExitStack

import concourse.bacc as bacc
import concourse.bass as bass
import concourse.tile as tile
import concourse.mybir as mybir
from concourse import masks
from concourse.bass_utils import run_bass_kernel_spmd

B, C, HW, TF, NH, CPH = 16, 512, 576, 512, 8, 64
NCORES, BPC = 8, B // 8
SCALE = 1.0 / 8.0  # 1/sqrt(CPH)
F32 = mybir.dt.float32
BF16 = mybir.dt.bfloat16
AF = mybir.ActivationFunctionType
OP = mybir.AluOpType
PD = 128
NCC = C // PD                                    # 4 channel chunks
MT = [(0, 116), (116, 115), (231, 115), (346, 115), (461, 115)]  # hw m-tiles
NHALF = [(0, 288), (288, 288)]                   # n halves (psum banks)
FHALF = [(0, 288), (288, 289)]                   # final-conv rhs halves
AVCH = [(0, 288), (288, 289)]                    # AV rhs chunks over es cols
CPS = 128                                        # per-head V'T stride
TMP = 8                                          # t_m_blk cols (one per head)
ESW = HW + 1                                     # es cols: 576 + cross col
SB = 512                                         # psum bank cols (f32)
SEQ = False


def _body(ctx: ExitStack, tc, d):
    """d: DRAM APs: x[2,512,576](bf16), t_m_blk[2,512,8](bf16),
    t2x2[2,512,1](f32), WqT/WkT/WvmT/WrT [512,512](bf16, [c_in,c_out]),
    Wr_b[512,1](f32), out[2,512,576](bf16)."""
    nc = tc.nc

    wt = ctx.enter_context(tc.tile_pool(name="wt", bufs=1))
    act = ctx.enter_context(tc.tile_pool(name="act", bufs=1))
    expp = ctx.enter_context(tc.tile_pool(name="expp", bufs=1))
    ps = ctx.enter_context(tc.tile_pool(name="ps", bufs=1, space="PSUM"))

    # ---- activation loads first (PE can start early), then weights ----
    xbts, tmbts, t2ts = [], [], []
    for b in range(BPC):
        xbts.append(act.tile([PD, NCC * HW], BF16, name=f"xb{b}", tag=f"xb{b}"))
        tmbts.append(act.tile([PD, NCC * TMP], BF16, name=f"tmblk{b}",
                              tag=f"tm{b}"))
        t2ts.append(act.tile([PD, NCC], F32, name=f"t2_{b}", tag=f"t2_{b}"))
    # x chunks split per-cc so the first conv group's input lands in ~0.5us
    for b in range(BPC):
        for cc in range(NCC):
            nc.sync.dma_start(
                xbts[b][:, cc * HW:(cc + 1) * HW],
                d["x"][b, cc * PD:(cc + 1) * PD, :])
        nc.sync.dma_start(tmbts[b][:].rearrange("p (cc h) -> p cc h", cc=NCC),
                          d["t_m_blk"][b].rearrange("(cc p) h -> p cc h", p=PD))
        nc.sync.dma_start(t2ts[b][:],
                          d["t2x2"][b].rearrange("(cc p) one -> p (cc one)", p=PD))
    W = {}
    for wn in ("WqT", "WkT", "WvmT", "WrT"):
        wtile = wt.tile([PD, NCC * C], BF16, name=f"{wn}_t")
        nc.scalar.dma_start(wtile[:].rearrange("p (cc o) -> p cc o", cc=NCC),
                            d[wn].rearrange("(cc p) o -> p cc o", p=PD))
        W[wn] = [wtile[:, j * C:(j + 1) * C] for j in range(NCC)]
    wrbt = wt.tile([PD, NCC], F32, name="wrbt")
    nc.sync.dma_start(wrbt[:], d["Wr_b"].rearrange("(cc p) one -> p (cc one)", p=PD))
    wrb = [wrbt[:, j:j + 1] for j in range(NCC)]
    ident = wt.tile([PD, PD], F32, name="ident")
    masks.make_identity(nc, ident[:])
    onesb = wt.tile([PD, SB], BF16, name="onesb")
    nc.vector.memset(onesb[:], 1.0)

    # persistent per-batch tiles (explicit double-buffering across batches)
    VT = [[wt.tile([sz, NH * CPS], BF16, name=f"vt{b}_{mi}")
           for mi, (m0, sz) in enumerate(MT)] for b in range(BPC)]
    for b in range(BPC):
        for mi, (m0, sz) in enumerate(MT):
            nc.gpsimd.memset(
                VT[b][mi][:].rearrange("p (h c) -> p h c", h=NH)[:, :, CPH:CPS],
                1.0)
    crossT = [[act.tile([sz, NH + 1], BF16, name=f"crossT{b}_{mi}",
                        tag=f"crossT{b}_{mi}")
               for mi, (m0, sz) in enumerate(MT)] for b in range(BPC)]
    QT = [[act.tile([PD, HW], BF16, name=f"q{b}_{j}", tag=f"q{b}_{j}")
           for j in range(NCC)] for b in range(BPC)]
    KT = [[act.tile([PD, HW], BF16, name=f"k{b}_{j}", tag=f"k{b}_{j}")
           for j in range(NCC)] for b in range(BPC)]
    OA = [[act.tile([PD, ESW], BF16, name=f"oa{b}_{j}", tag=f"oa{b}_{j}")
           for j in range(NCC)] for b in range(BPC)]
    FIN = [[act.tile([PD, HW], BF16, name=f"fin{b}_{j}", tag=f"fin{b}_{j}")
            for j in range(NCC)] for b in range(BPC)]
    FA = [[act.tile([PD, HW], BF16, name=f"fa{b}_{j}", tag=f"fa{b}_{j}")
           for j in range(NCC)] for b in range(BPC)]
    # softmax-denominator scratch, alternated by head-pair parity:
    # chunks parked at partitions 0/32 (sub0) and 64/96 (sub1)
    smt2 = [wt.tile([97, 288], F32, name=f"smt{i}") for i in range(2)]
    smr2 = [wt.tile([97, 288], F32, name=f"smr{i}") for i in range(2)]
    smb2 = [[wt.tile([1, 288], F32, name=f"smb{i}_{k}") for k in range(3)]
            for i in range(2)]
    rep2 = [[wt.tile([CPH, HW], F32, name=f"rep{i}_{s}") for s in range(2)]
            for i in range(2)]

    def xb(b, j):
        return xbts[b][:, j * HW:(j + 1) * HW]

    evict_rr = [0]

    def evict(engines, out_ap, in_ap):
        e = engines[evict_rr[0] % len(engines)]
        evict_rr[0] += 1
        if e == "v":
            nc.vector.tensor_copy(out_ap, in_ap)
        else:
            nc.scalar.copy(out_ap, in_ap)

    def conv_ot_gen(b, Wn, outs, ot, engines):
        p = ps.tile([PD, 2 * SB], F32, tag="s2", bufs=2, name=f"p_c{b}_{ot}")
        for hi, (n0, nsz) in enumerate(NHALF):
            for cc in range(NCC):
                nc.tensor.matmul(
                    p[:, hi * SB:hi * SB + nsz],
                    Wn[cc][:, ot * PD:(ot + 1) * PD],
                    xb(b, cc)[:, n0:n0 + nsz],
                    start=(cc == 0), stop=(cc == NCC - 1))
            yield
        evict(engines,
              outs[ot][:, 0:HW].rearrange("p (h n) -> p h n", h=2),
              p[:].rearrange("p (h n) -> p h n", h=2)[:, :, 0:288])

    def conv_gen(b, Wn, outs, engines):
        for ot in range(NCC):
            yield from conv_ot_gen(b, Wn, outs, ot, engines)

    def vt_gen(b, engines):
        for mi, (m0, sz) in enumerate(MT):
            p = ps.tile([sz, SB], F32, tag="s2", bufs=2, name=f"p_vt{b}_{mi}")
            for cc in range(NCC):
                nc.tensor.matmul(p[:], xb(b, cc)[:, m0:m0 + sz],
                                 W["WvmT"][cc][:],
                                 start=(cc == 0), stop=(cc == NCC - 1))
                if cc == 1:
                    yield
            yield
            vv = VT[b][mi][:].rearrange("p (h c) -> p h c", h=NH)
            evict(engines, vv[:, :, 0:CPH],
                  p[:].rearrange("p (h c) -> p h c", h=NH))

    def cross_gen(b):
        tmblk = tmbts[b]
        p = ps.tile([TMP, 2 * SB], F32, tag="s2", bufs=2, name=f"p_cl{b}")
        for hi, (n0, nsz) in enumerate(NHALF):
            for cc in range(NCC):
                nc.tensor.matmul(p[0:TMP, hi * SB:hi * SB + nsz],
                                 tmblk[:, cc * TMP:(cc + 1) * TMP],
                                 xb(b, cc)[:, n0:n0 + nsz],
                                 start=(cc == 0), stop=(cc == NCC - 1))
            yield
        crosse = act.tile([NH, HW], F32, name=f"crosse{b}", tag=f"crosse{b}")
        csum = [act.tile([NH, 1], F32, name=f"csum{b}_{i}", tag=f"csum{b}_{i}")
                for i in range(2)]
        for hi, (n0, nsz) in enumerate(NHALF):
            nc.scalar.activation(crosse[:, n0:n0 + nsz],
                                 p[0:NH, hi * SB:hi * SB + nsz],
                                 AF.Exp, scale=SCALE, accum_out=csum[hi][:])
        crec = act.tile([NH, 1], F32, name=f"crec{b}", tag=f"crec{b}")
        nc.vector.tensor_add(crec[:], csum[0][:], csum[1][:])
        nc.vector.reciprocal(crec[:], crec[:])
        crossn = act.tile([NH, HW], F32, name=f"crossn{b}", tag=f"crossn{b}")
        nc.vector.tensor_scalar_mul(crossn[:], crosse[:], crec[:])
        for mi, (m0, sz) in enumerate(MT):
            pt = ps.tile([sz, NH], F32, tag="s2", bufs=2, name=f"p_ct{b}_{mi}")
            nc.tensor.transpose(pt[:], crossn[:, m0:m0 + sz],
                                ident[0:NH, 0:NH])
            nc.vector.tensor_copy(crossT[b][mi][0:sz, 0:NH], pt[:])
            if mi % 2 == 1:
                yield

    def final_ot_gen(b, ot, half):
        # K-split final conv: half 0 contracts cc 0..1 (ready after hp 1),
        # half 1 contracts cc 2..3 and merges with half 0's partial.
        ccs = (0, 1) if half == 0 else (2, 3)
        p = ps.tile([PD, 2 * SB], F32, tag="s2", bufs=2,
                    name=f"p_f{b}_{ot}_{half}")
        for hi, (n0, nsz) in enumerate(FHALF):
            for cc in ccs:
                nc.tensor.matmul(
                    p[:, hi * SB:hi * SB + nsz],
                    W["WrT"][cc][:, ot * PD:(ot + 1) * PD],
                    OA[b][cc][:, n0:n0 + nsz],
                    start=(cc == ccs[0]), stop=(cc == ccs[1]))
            yield
        # col 800 = Wr @ (crossAV + 2*t2) partial: add to every column
        if half == 0:
            nc.vector.tensor_scalar(
                FA[b][ot][:].rearrange("p (h n) -> p h n", h=2),
                p[:].rearrange("p (h n) -> p h n", h=2)[:, :, 0:288],
                wrb[ot], p[:, SB + 288:SB + 289], OP.add, OP.add)
        else:
            nc.vector.scalar_tensor_tensor(
                FIN[b][ot][:].rearrange("p (h n) -> p h n", h=2),
                p[:].rearrange("p (h n) -> p h n", h=2)[:, :, 0:288],
                p[:, SB + 288:SB + 289],
                FA[b][ot][:].rearrange("p (h n) -> p h n", h=2),
                OP.add, OP.add)
            nc.sync.dma_start(d["out"][b, ot * PD:(ot + 1) * PD, :],
                              FIN[b][ot][:])

    def final_gen(b):
        for half in range(2):
            for ot in range(NCC):
                yield from final_ot_gen(b, ot, half)

    def pair_gen(b, hp):
        K, Q = KT[b][hp], QT[b][hp]
        h2 = (2 * hp, 2 * hp + 1)
        es = [[expp.tile([sz, ESW], BF16, name=f"es{b}_{hp}_{sub}_{mi}",
                         tag=f"es{sub}_{mi}", bufs=3)
               for mi, (m0, sz) in enumerate(MT)] for sub in range(2)]
        for mi, (m0, sz) in enumerate(MT):
            for sub in range(2):
                rr = sub * CPH
                p = ps.tile([sz, 2 * SB], F32, tag="s2", bufs=2,
                            name=f"p_s{b}_{hp}_{sub}_{mi}")
                for hi, (n0, nsz) in enumerate(NHALF):
                    nc.tensor.matmul(
                        p[:, hi * SB:hi * SB + nsz],
                        K[rr:rr + CPH, m0:m0 + sz],
                        Q[rr:rr + CPH, n0:n0 + nsz],
                        start=True, stop=True, tile_position=(rr, 0),
                        skip_group_check=True)
                yield
                nc.scalar.activation(
                    es[sub][mi][:, 0:HW].rearrange("p (h n) -> p h n", h=2),
                    p[:].rearrange("p (h n) -> p h n", h=2)[:, :, 0:288],
                    AF.Exp, scale=SCALE)
                nc.gpsimd.tensor_copy(
                    es[sub][mi][:, HW:ESW],
                    crossT[b][mi][0:sz, h2[sub]:h2[sub] + 1])
        par = hp % 2
        smt, smr, smb, rep = smt2[par], smr2[par], smb2[par], rep2[par]
        pavs = []
        for sub in range(2):
            h = h2[sub]
            pav = ps.tile([PD, 2 * SB], F32, tag="av", bufs=2,
                          name=f"p_av{b}_{h}")
            pavs.append(pav)
            for mi, (m0, sz) in enumerate(MT):
                lhs = VT[b][mi][:, h * CPS:(h + 1) * CPS]
                st, sp = (mi == 0), (mi == len(MT) - 1)
                for ci, (c0, nsz) in enumerate(AVCH):
                    nc.tensor.matmul(pav[:, ci * SB:ci * SB + nsz], lhs,
                                     es[sub][mi][:, c0:c0 + nsz],
                                     start=st, stop=sp)
            # park the two denominator chunks at partitions 64*sub, 64*sub+32
            q0 = 64 * sub
            nc.vector.tensor_copy(smt[q0:q0 + 1, :], pav[CPH:CPH + 1, 0:288])
            nc.vector.tensor_copy(smt[q0 + 32:q0 + 33, :],
                                  pav[CPH:CPH + 1, SB:SB + 288])
        nc.vector.reciprocal(smr[:], smt[:])
        for k in range(3):
            nc.vector.tensor_copy(smb[k][:], smr[32 * (k + 1):32 * (k + 1) + 1, :])
        nc.gpsimd.partition_broadcast(rep[0][:, 0:288], smr[0:1, :])
        nc.gpsimd.partition_broadcast(rep[0][:, 288:HW], smb[0][:])
        nc.gpsimd.partition_broadcast(rep[1][:, 0:288], smb[1][:])
        nc.gpsimd.partition_broadcast(rep[1][:, 288:HW], smb[2][:])
        yield
        for sub in range(2):
            rr = sub * CPH
            pav = pavs[sub]
            dst = OA[b][hp][rr:rr + CPH, :]
            nc.vector.tensor_tensor(
                dst[:, 0:HW].rearrange("p (h n) -> p h n", h=2),
                pav[0:CPH, :].rearrange("p (h n) -> p h n", h=2)[:, :, 0:288],
                rep[sub][:].rearrange("p (h n) -> p h n", h=2), OP.mult)
            # col 576: cross-attention value + pass-through V bias (2*t2)
            nc.vector.tensor_scalar_add(dst[:, HW:ESW],
                                        pav[0:CPH, SB + 288:SB + 289],
                                        t2ts[b][rr:rr + CPH, hp:hp + 1])
            yield

    # ---- driver: weave filler chunks between pair chunks ----
    fillers = deque()

    def step_fill():
        while fillers:
            try:
                next(fillers[0])
                return
            except StopIteration:
                fillers.popleft()

    def drive(gen):
        for _ in gen:
            step_fill()

    def drain():
        while fillers:
            try:
                next(fillers[0])
            except StopIteration:
                fillers.popleft()

    EV0 = ("v", "a")        # batch-0 conv phase: ACT is idle, share with DVE
    EV1 = ("v",)            # fillers during heads: ACT is busy with exp

    if SEQ:
        for b in range(BPC):
            drive(conv_gen(b, W["WqT"], QT[b], EV0))
            drive(conv_gen(b, W["WkT"], KT[b], EV0))
            drive(vt_gen(b, EV0))
            drive(cross_gen(b))
            for hp in range(NCC):
                drive(pair_gen(b, hp))
            drive(final_gen(b))
        return

    # phase 1: batch-0 convs (PE-dense, no fillers yet)
    drive(conv_gen(0, W["WqT"], QT[0], EV0))
    drive(conv_gen(0, W["WkT"], KT[0], EV0))
    drive(vt_gen(0, EV0))
    drive(cross_gen(0))
    # phase 2: batch-0 heads, batch-1 convs woven in as fillers
    fillers.append(conv_ot_gen(1, W["WqT"], QT[1], 0, EV1))
    fillers.append(conv_ot_gen(1, W["WkT"], KT[1], 0, EV1))
    drive(pair_gen(0, 0))
    fillers.append(vt_gen(1, EV1))
    drive(pair_gen(0, 1))
    fillers.append(conv_ot_gen(1, W["WqT"], QT[1], 1, EV1))
    fillers.append(conv_ot_gen(1, W["WkT"], KT[1], 1, EV1))
    drive(pair_gen(0, 2))
    fillers.append(cross_gen(1))
    fillers.append(final_ot_gen(0, 0, 0))
    fillers.append(final_ot_gen(0, 1, 0))
    drive(pair_gen(0, 3))
    drain()
    # phase 3: batch-1 heads, deferred batch-1 convs + batch-0 final as fillers
    fillers.append(final_ot_gen(0, 2, 0))
    fillers.append(final_ot_gen(0, 3, 0))
    fillers.append(conv_ot_gen(1, W["WqT"], QT[1], 2, EV1))
    fillers.append(conv_ot_gen(1, W["WkT"], KT[1], 2, EV1))
    drive(pair_gen(1, 0))
    fillers.append(conv_ot_gen(1, W["WqT"], QT[1], 3, EV1))
    fillers.append(conv_ot_gen(1, W["WkT"], KT[1], 3, EV1))
    fillers.append(final_ot_gen(0, 0, 1))
    fillers.append(final_ot_gen(0, 1, 1))
    drive(pair_gen(1, 1))
    fillers.append(final_ot_gen(0, 2, 1))
    fillers.append(final_ot_gen(0, 3, 1))
    drive(pair_gen(1, 2))
    for ot in range(NCC):
        fillers.append(final_ot_gen(1, ot, 0))
    drive(pair_gen(1, 3))
    drain()
    # tail: batch-1 final conv second K-half
    for ot in range(NCC):
        drive(final_ot_gen(1, ot, 1))


_CACHE = {}


def _build():
    if "nc" in _CACHE:
        return _CACHE["nc"], _CACHE["out"]
    nc = bacc.Bacc("TRN2", target_bir_lowering=False, debug=False,
                   num_devices=NCORES)
    d = {
        "x": nc.dram_tensor("x", [BPC, C, HW], BF16, kind="ExternalInput").ap(),
        "t_m_blk": nc.dram_tensor("t_m_blk", [BPC, C, TMP], BF16,
                                  kind="ExternalInput").ap(),
        "t2x2": nc.dram_tensor("t2x2", [BPC, C, 1], F32,
                               kind="ExternalInput").ap(),
        "Wr_b": nc.dram_tensor("Wr_b", [C, 1], F32, kind="ExternalInput").ap(),
        "out": nc.dram_tensor("out", [BPC, C, HW], BF16,
                              kind="ExternalOutput").ap(),
    }
    for wn in ("WqT", "WkT", "WvmT", "WrT"):
        d[wn] = nc.dram_tensor(wn, [C, C], BF16, kind="ExternalInput").ap()
    with tile.TileContext(nc) as tc:
        with ExitStack() as ctx:
            _body(ctx, tc, d)
    nc.compile()
    _CACHE["nc"], _CACHE["out"] = nc, d["out"].tensor.name
    return nc, _CACHE["out"]


def _prep_inputs(x, t, Wk, Wq, Wt_w, Wt_b, Wm, Wv, Wr_w, Wr_b):
    f = np.float32
    bf = ml_dtypes.bfloat16
    x = np.asarray(x, f).reshape(B, C, HW)
    t = np.asarray(t, f)
    Wm1 = np.asarray(Wm, f)[:, :C]
    Wm2 = np.asarray(Wm, f)[:, C:]
    Wv = np.asarray(Wv, f)
    t_m = t @ np.asarray(Wt_w, f).T + np.asarray(Wt_b, f)
    t_m_blk = np.zeros((B, C, TMP), f)
    for h in range(NH):
        t_m_blk[:, h * CPH:(h + 1) * CPH, h] = t_m[:, h * CPH:(h + 1) * CPH]
    t2x2 = (2.0 * (t @ Wm2.T @ Wv.T)).reshape(B, C, 1)
    com = {
        "WqT": np.ascontiguousarray(np.asarray(Wq, f).T).astype(bf),
        "WkT": np.ascontiguousarray(np.asarray(Wk, f).T).astype(bf),
        "WvmT": np.ascontiguousarray((Wv @ Wm1).T).astype(bf),
        "WrT": np.ascontiguousarray(np.asarray(Wr_w, f).T).astype(bf),
        "Wr_b": np.asarray(Wr_b, f).reshape(C, 1),
    }
    maps = []
    for c in range(NCORES):
        sl = slice(c * BPC, (c + 1) * BPC)
        m = dict(com)
        m["x"] = np.ascontiguousarray(x[sl]).astype(bf)
        m["t_m_blk"] = np.ascontiguousarray(t_m_blk[sl]).astype(bf)
        m["t2x2"] = np.ascontiguousarray(t2x2[sl])
        maps.append(m)
    return maps


def kernel(x, t, Wk, Wq, Wt_w, Wt_b, Wm, Wv, Wr_w, Wr_b, _trace=False):
    nc, out_name = _build()
    maps = _prep_inputs(x, t, Wk, Wq, Wt_w, Wt_b, Wm, Wv, Wr_w, Wr_b)
    res = run_bass_kernel_spmd(nc, maps, core_ids=list(range(NCORES)),
                               trace=_trace)
    out = np.concatenate(
        [np.asarray(res.results[c][out_name], dtype=np.float32)
         for c in range(NCORES)], axis=0).reshape(B, C, 24, 24)
    if _trace:
        kernel.last_results = res
    return out
